# revision 5
# baseline (speedup 1.0000x reference)
"""CAML attention kernel for Trainium2 (8 NeuronCores, SPMD over batch).

Reference computation:
    xt      = tanh(x)                      # [B, D, L]
    scores  = einsum('cd,bdl->bcl', W1, xt)
    weights = softmax(scores, axis=l)
    weighted= einsum('bcl,bdl->bcd', weights, xt)
    out     = einsum('cd,bcd->bc', W2, weighted) + b2

Key identity: the final contraction commutes with the softmax weighted sum,
so with s2 = einsum('cd,bdl->bcl', W2, xt):
    out[b,c] = (sum_l exp(s1)*s2) / (sum_l exp(s1)) + b2
(|s1| <= 512*max|W1| ~ 13, so exp without max-subtraction is safe in fp32.)

v2 design (vs the 637us fp16 C-sharded baseline):
  * Batch-sharded: core i computes batch i with the full class range
    (C padded 8930 -> 8960 = 70*128). 8x less x DMA + tanh per core, and
    jch drops 72 -> 70 vs the C_PAD=9216 C-sharding.
  * Both matmuls in fp8-e4m3 DoubleRow (contraction 256/instr): weights are
    scaled by 16 into e4m3's normal range; exp() compensates with scale=1/16
    and the numerator product with scalar=1/16. tanh() writes fp8 directly
    from ACT (no DVE cast pass). Measured CPU-sim rel err 1.5e-2 (< 2e-2).
  * PSUM groups of 1024 cols (2 banks; L = 1024+1024+452): 3 exp + 3
    product ops per j instead of 5, fewer fixed overheads + accumulator
    reads on ACT/DVE (ACT was 100% busy in the fp8-s1 experiment).
  * Batched epilogue: per-(j,lc) numer/denom partials land in persistent
    [P, 210] accumulators via ACT/DVE accum_out; one segmented reduce +
    reciprocal + 2 elementwise ops at the end replace 4-6 small ops per j.
"""

import numpy as np
import ml_dtypes

import concourse.bacc as bacc
import concourse.tile as tile
from concourse import mybir
from concourse.bass_utils import run_bass_kernel_spmd

B, D, L, C = 8, 512, 2500, 8930
N_CORES = 8
P = 128

C_PAD = 8960                 # next multiple of 128 above C
JCH = C_PAD // P             # 70 class chunks per core
KCH = D // P                 # 4 contraction chunks (2 DoubleRow pairs)
JCW = [4, 11, 11, 11, 11, 11, 11]   # j's per weight-DMA chunk (small first)
JCO = [0, 4, 15, 26, 37, 48, 59]     # chunk offsets
NJC = len(JCW)
LCS = [(0, 1024), (1024, 1024), (2048, 452)]   # (start, len) PSUM groups
LCW = [1024, 1024, 512]      # xt8 tile row strides (16B-aligned for fp8 rhs)

F32 = mybir.dt.float32
BF16 = mybir.dt.bfloat16
FP8 = mybir.dt.float8e4
FP8_NP = mybir.dt.np(mybir.dt.float8e4)   # ml_dtypes.float8_e4m3
BF16_NP = ml_dtypes.bfloat16

W_SCALE = 16.0               # lift ~U(-0.025, 0.025) weights into e4m3 normals
DR = mybir.MatmulPerfMode.DoubleRow


def build_nc():
    """Emit the per-core program. All cores run the same NEFF (SPMD)."""
    nc = bacc.Bacc("TRN2", target_bir_lowering=False, debug=False)

    x = nc.dram_tensor("x", [P, KCH, L], BF16, kind="ExternalInput")
    w1t = nc.dram_tensor("w1t", [P, KCH, C_PAD], FP8, kind="ExternalInput")
    w2t = nc.dram_tensor("w2t", [P, KCH, C_PAD], FP8, kind="ExternalInput")
    b2s = nc.dram_tensor("b2s", [P, JCH], F32, kind="ExternalInput")
    out = nc.dram_tensor("out", [P, JCH], F32, kind="ExternalOutput")

    Exp = mybir.ActivationFunctionType.Exp
    Tanh = mybir.ActivationFunctionType.Tanh
    mult = mybir.AluOpType.mult
    add = mybir.AluOpType.add

    with tile.TileContext(nc) as tc:
        with (
            tc.tile_pool(name="wts", bufs=1) as wpool,
            tc.tile_pool(name="xraw", bufs=1) as xpool,
            tc.tile_pool(name="xt8", bufs=1) as xtpool,
            tc.tile_pool(name="ps1", bufs=2, space="PSUM") as ppool1,
            tc.tile_pool(name="ps2", bufs=2, space="PSUM") as ppool2,
            tc.tile_pool(name="etile", bufs=4) as epool,
            tc.tile_pool(name="prod", bufs=3) as spool,
            tc.tile_pool(name="acc", bufs=1) as apool,
        ):
            # Weight SBUF tiles, one per DMA chunk so early matmuls only
            # depend on the first chunk's arrival.
            w1sb = [wpool.tile([P, KCH, JCW[ci] * P], FP8, tag=f"w1_{ci}", name=f"w1sb{ci}") for ci in range(NJC)]
            w2sb = [wpool.tile([P, KCH, JCW[ci] * P], FP8, tag=f"w2_{ci}", name=f"w2sb{ci}") for ci in range(NJC)]
            b2sb = wpool.tile([P, JCH], F32, tag="b2")

            # x chunks (bf16) and tanh(x) in fp8; chunk 0 split in halves
            # so the very first matmuls wait on only 256 KB of x
            XCH = [(0, 512), (512, 512), (1024, 1024), (2048, 452)]
            xraw = [xpool.tile([P, KCH, lw], BF16, tag=f"xr_{i}", name=f"xraw{i}") for i, (_, lw) in enumerate(XCH)]
            xt8 = [xtpool.tile([P, KCH, LCW[i]], FP8, tag=f"xt_{i}", name=f"xt8_{i}") for i in range(3)]

            # numer/denom partial accumulators: col j*3 + lc
            dall = apool.tile([P, 3 * JCH], F32, tag="dall")
            nall = apool.tile([P, 3 * JCH], F32, tag="nall")

            # DMA order = first-consumption order on the single sync queue
            def wslice(ci):
                return slice(JCO[ci] * P, (JCO[ci] + JCW[ci]) * P)

            nc.sync.dma_start(out=xraw[0], in_=x[:, :, 0:512])
            nc.sync.dma_start(out=w1sb[0], in_=w1t[:, :, wslice(0)])
            nc.sync.dma_start(out=w2sb[0], in_=w2t[:, :, wslice(0)])
            nc.sync.dma_start(out=xraw[1], in_=x[:, :, 512:1024])
            nc.sync.dma_start(out=xraw[2], in_=x[:, :, 1024:2048])
            nc.sync.dma_start(out=xraw[3], in_=x[:, :, 2048:2500])
            for ci in range(1, NJC):
                nc.sync.dma_start(out=w1sb[ci], in_=w1t[:, :, wslice(ci)])
                nc.sync.dma_start(out=w2sb[ci], in_=w2t[:, :, wslice(ci)])
            nc.sync.dma_start(out=b2sb, in_=b2s[:])

            for j in range(JCH):
                ci = next(i for i in range(NJC - 1, -1, -1) if JCO[i] <= j)
                jl = j - JCO[ci]
                for lc, (ls, lw) in enumerate(LCS):
                    if j == 0:
                        # tanh for this l-chunk, emitted right before its
                        # first consumer so ACT doesn't head-block on the
                        # later x DMAs.
                        for xi, (xs, xw) in enumerate(XCH):
                            if ls <= xs < ls + lw:
                                nc.scalar.activation(
                                    out=xt8[lc][:, :, xs - ls : xs - ls + xw],
                                    in_=xraw[xi], func=Tanh,
                                )
                    s1 = ppool1.tile([P, 1024], F32)
                    s2 = ppool2.tile([P, 1024], F32)
                    ncg = (lw + 511) // 512
                    for wsb, s in ((w1sb, s1), (w2sb, s2)):
                        for cg in range(ncg):
                            a, b = 512 * cg, min(512 * (cg + 1), lw)
                            for pr in range(KCH // 2):
                                nc.tensor.matmul(
                                    s[:, a:b],
                                    wsb[ci][:, 2 * pr : 2 * pr + 2, jl * P : (jl + 1) * P],
                                    xt8[lc][:, 2 * pr : 2 * pr + 2, a:b],
                                    start=(pr == 0),
                                    stop=(pr == KCH // 2 - 1),
                                    perf_mode=DR,
                                )
                    idx = 3 * j + lc
                    e = epool.tile([P, 1024], F32)
                    nc.scalar.activation(
                        out=e[:, :lw], in_=s1[:, :lw], func=Exp,
                        scale=1.0 / W_SCALE,
                        accum_out=dall[:, idx : idx + 1],
                    )
                    prod = spool.tile([P, 1024], F32)
                    # numer partial = sum_l E * (s2/16), single DVE pass
                    nc.vector.scalar_tensor_tensor(
                        out=prod[:, :lw], in0=e[:, :lw], scalar=1.0 / W_SCALE,
                        in1=s2[:, :lw], op0=mult, op1=mult,
                        accum_out=nall[:, idx : idx + 1],
                    )

            # Batched epilogue: segmented reduce over the 3 partials per j,
            # then out = numer/denom + b2 elementwise over [P, JCH].
            #
            # dall is written by ACT's accumulator-read micro-ops, and the
            # dependency tracker misses that cross-engine edge (observed: the
            # dred reduce issues with no ACT-side wait and intermittently
            # reads stale data). Force the ordering explicitly: a trailing
            # ACT copy is FIFO-ordered after every accumulator write, and a
            # DVE read of its (tracked) main output pins the DVE queue
            # behind it.
            probe = apool.tile([P, 1], F32, tag="probe")
            nc.scalar.activation(
                out=probe, in_=dall[:, 3 * JCH - 1 : 3 * JCH],
                func=mybir.ActivationFunctionType.Copy,
            )
            probe2 = apool.tile([P, 1], F32, tag="probe2")
            nc.vector.tensor_copy(probe2, probe)
            dred = apool.tile([P, JCH], F32, tag="dred")
            nred = apool.tile([P, JCH], F32, tag="nred")
            recip = apool.tile([P, JCH], F32, tag="recip")
            quot = apool.tile([P, JCH], F32, tag="quot")
            osb = apool.tile([P, JCH], F32, tag="osb")
            AX = mybir.AxisListType.X
            nc.vector.tensor_reduce(
                out=dred, in_=dall.rearrange("p (j l) -> p j l", l=3),
                axis=AX, op=add,
            )
            nc.vector.tensor_reduce(
                out=nred, in_=nall.rearrange("p (j l) -> p j l", l=3),
                axis=AX, op=add,
            )
            nc.vector.reciprocal(recip, dred)
            nc.vector.scalar_tensor_tensor(
                out=quot, in0=nred, scalar=1.0, in1=recip, op0=mult, op1=mult
            )
            nc.vector.scalar_tensor_tensor(
                out=osb, in0=quot, scalar=1.0, in1=b2sb, op0=mult, op1=add
            )
            nc.sync.dma_start(out=out[:], in_=osb)

    nc.compile()
    return nc


_NC_CACHE = {}


def _get_nc():
    if "nc" not in _NC_CACHE:
        _NC_CACHE["nc"] = build_nc()
    return _NC_CACHE["nc"]


def make_in_maps(x, W1, W2, b2):
    """Host-side prep: pad C, transpose + scale + fp8-cast weights, bf16 x."""
    x = np.asarray(x, dtype=np.float32)

    def prep_w(W):
        Wp = np.zeros((C_PAD, D), dtype=np.float32)
        Wp[:C] = np.asarray(W, dtype=np.float32)
        # [C_PAD, D] -> [P(d within chunk), KCH, C_PAD]
        return np.ascontiguousarray(
            Wp.T.reshape(KCH, P, C_PAD).transpose(1, 0, 2) * W_SCALE
        ).astype(FP8_NP)

    w1c, w2c = prep_w(W1), prep_w(W2)
    b2p = np.zeros((C_PAD,), dtype=np.float32)
    b2p[:C] = np.asarray(b2, dtype=np.float32)
    b2c = np.ascontiguousarray(b2p.reshape(JCH, P).T)

    in_maps = []
    for i in range(N_CORES):
        xc = np.ascontiguousarray(
            x[i].reshape(KCH, P, L).transpose(1, 0, 2)
        ).astype(BF16_NP)
        in_maps.append({"x": xc, "w1t": w1c, "w2t": w2c, "b2s": b2c})
    return in_maps


def gather_out(results):
    """results: list (per core) of {'out': [P, JCH]} -> full [B, C]."""
    parts = [
        np.asarray(r["out"], dtype=np.float32).T.reshape(C_PAD)[:C]
        for r in results
    ]
    return np.stack(parts, axis=0)


def kernel(x, W1, W2, b2):
    nc = _get_nc()
    in_maps = make_in_maps(x, W1, W2, b2)
    res = run_bass_kernel_spmd(nc, in_maps, list(range(N_CORES)))
    return gather_out(res.results)


# revision 6
# speedup vs baseline: 1.0340x; 1.0340x over previous
"""CAML attention kernel for Trainium2 (8 NeuronCores, SPMD over batch).

Reference computation:
    xt      = tanh(x)                      # [B, D, L]
    scores  = einsum('cd,bdl->bcl', W1, xt)
    weights = softmax(scores, axis=l)
    weighted= einsum('bcl,bdl->bcd', weights, xt)
    out     = einsum('cd,bcd->bc', W2, weighted) + b2

Key identity: the final contraction commutes with the softmax weighted sum,
so with s2 = einsum('cd,bdl->bcl', W2, xt):
    out[b,c] = (sum_l exp(s1)*s2) / (sum_l exp(s1)) + b2
(|s1| <= 512*max|W1| ~ 13, so exp without max-subtraction is safe in fp32.)

v2 design (vs the 637us fp16 C-sharded baseline):
  * Batch-sharded: core i computes batch i with the full class range
    (C padded 8930 -> 8960 = 70*128). 8x less x DMA + tanh per core, and
    jch drops 72 -> 70 vs the C_PAD=9216 C-sharding.
  * Both matmuls in fp8-e4m3 DoubleRow (contraction 256/instr): weights are
    scaled by 16 into e4m3's normal range; exp() compensates with scale=1/16
    and the numerator product with scalar=1/16. tanh() writes fp8 directly
    from ACT (no DVE cast pass). Measured CPU-sim rel err 1.5e-2 (< 2e-2).
  * PSUM groups of 1024 cols (2 banks; L = 1024+1024+452): 3 exp + 3
    product ops per j instead of 5, fewer fixed overheads + accumulator
    reads on ACT/DVE (ACT was 100% busy in the fp8-s1 experiment).
  * Batched epilogue: per-(j,lc) numer/denom partials land in persistent
    [P, 210] accumulators via ACT/DVE accum_out; one segmented reduce +
    reciprocal + 2 elementwise ops at the end replace 4-6 small ops per j.
"""

import numpy as np
import ml_dtypes

import concourse.bacc as bacc
import concourse.tile as tile
from concourse import mybir
from concourse.bass_utils import run_bass_kernel_spmd

B, D, L, C = 8, 512, 2500, 8930
N_CORES = 8
P = 128

C_PAD = 8960                 # next multiple of 128 above C
JCH = C_PAD // P             # 70 class chunks per core
KCH = D // P                 # 4 contraction chunks (2 DoubleRow pairs)
JCW = [4, 11, 11, 11, 11, 11, 11]   # j's per weight-DMA chunk (small first)
JCO = [0, 4, 15, 26, 37, 48, 59]     # chunk offsets
NJC = len(JCW)
LCS = [(0, 1024), (1024, 1024), (2048, 452)]   # (start, len) PSUM groups
LCW = [1024, 1024, 512]      # xt8 tile row strides (16B-aligned for fp8 rhs)

F32 = mybir.dt.float32
BF16 = mybir.dt.bfloat16
FP8 = mybir.dt.float8e4
FP8_NP = mybir.dt.np(mybir.dt.float8e4)   # ml_dtypes.float8_e4m3
BF16_NP = ml_dtypes.bfloat16

W_SCALE = 16.0               # lift ~U(-0.025, 0.025) weights into e4m3 normals
DR = mybir.MatmulPerfMode.DoubleRow


def build_nc():
    """Emit the per-core program. All cores run the same NEFF (SPMD)."""
    nc = bacc.Bacc("TRN2", target_bir_lowering=False, debug=False)

    x = nc.dram_tensor("x", [P, KCH, L], BF16, kind="ExternalInput")
    w1t = nc.dram_tensor("w1t", [P, KCH, C_PAD], FP8, kind="ExternalInput")
    w2t = nc.dram_tensor("w2t", [P, KCH, C_PAD], FP8, kind="ExternalInput")
    b2s = nc.dram_tensor("b2s", [P, JCH], F32, kind="ExternalInput")
    out = nc.dram_tensor("out", [P, JCH], F32, kind="ExternalOutput")

    Exp = mybir.ActivationFunctionType.Exp
    Tanh = mybir.ActivationFunctionType.Tanh
    mult = mybir.AluOpType.mult
    add = mybir.AluOpType.add

    with tile.TileContext(nc) as tc:
        with (
            tc.tile_pool(name="wts", bufs=1) as wpool,
            tc.tile_pool(name="xraw", bufs=1) as xpool,
            tc.tile_pool(name="xt8", bufs=1) as xtpool,
            tc.tile_pool(name="ps1", bufs=2, space="PSUM") as ppool1,
            tc.tile_pool(name="ps2", bufs=4, space="PSUM") as ppool2,
            tc.tile_pool(name="etile", bufs=4) as epool,
            tc.tile_pool(name="prod", bufs=3) as spool,
            tc.tile_pool(name="acc", bufs=1) as apool,
        ):
            # Weight SBUF tiles, one per DMA chunk so early matmuls only
            # depend on the first chunk's arrival.
            w1sb = [wpool.tile([P, KCH, JCW[ci] * P], FP8, tag=f"w1_{ci}", name=f"w1sb{ci}") for ci in range(NJC)]
            w2sb = [wpool.tile([P, KCH, JCW[ci] * P], FP8, tag=f"w2_{ci}", name=f"w2sb{ci}") for ci in range(NJC)]
            b2sb = wpool.tile([P, JCH], F32, tag="b2")

            # x chunks (bf16) and tanh(x) in fp8; chunk 0 split in halves
            # so the very first matmuls wait on only 256 KB of x
            XCH = [(0, 512), (512, 512), (1024, 1024), (2048, 452)]
            xraw = [xpool.tile([P, KCH, lw], BF16, tag=f"xr_{i}", name=f"xraw{i}") for i, (_, lw) in enumerate(XCH)]
            xt8 = [xtpool.tile([P, KCH, LCW[i]], FP8, tag=f"xt_{i}", name=f"xt8_{i}") for i in range(3)]

            # partial accumulators: denom col j*3 + lc, numer col j*5 + cg
            dall = apool.tile([P, 3 * JCH], F32, tag="dall")
            nall = apool.tile([P, 5 * JCH], F32, tag="nall")

            # DMA order = first-consumption order on the single sync queue
            def wslice(ci):
                return slice(JCO[ci] * P, (JCO[ci] + JCW[ci]) * P)

            nc.sync.dma_start(out=xraw[0], in_=x[:, :, 0:512])
            nc.sync.dma_start(out=w1sb[0], in_=w1t[:, :, wslice(0)])
            nc.sync.dma_start(out=w2sb[0], in_=w2t[:, :, wslice(0)])
            nc.sync.dma_start(out=xraw[1], in_=x[:, :, 512:1024])
            nc.sync.dma_start(out=xraw[2], in_=x[:, :, 1024:2048])
            nc.sync.dma_start(out=xraw[3], in_=x[:, :, 2048:2500])
            for ci in range(1, NJC):
                nc.sync.dma_start(out=w1sb[ci], in_=w1t[:, :, wslice(ci)])
                nc.sync.dma_start(out=w2sb[ci], in_=w2t[:, :, wslice(ci)])
            nc.sync.dma_start(out=b2sb, in_=b2s[:])

            for j in range(JCH):
                ci = next(i for i in range(NJC - 1, -1, -1) if JCO[i] <= j)
                jl = j - JCO[ci]
                for lc, (ls, lw) in enumerate(LCS):
                    if j == 0:
                        # tanh for this l-chunk, emitted right before its
                        # first consumer so ACT doesn't head-block on the
                        # later x DMAs.
                        for xi, (xs, xw) in enumerate(XCH):
                            if ls <= xs < ls + lw:
                                nc.scalar.activation(
                                    out=xt8[lc][:, :, xs - ls : xs - ls + xw],
                                    in_=xraw[xi], func=Tanh,
                                )
                    s1 = ppool1.tile([P, 1024], F32)
                    ncg = (lw + 511) // 512
                    cgs = [(512 * cg, min(512 * (cg + 1), lw)) for cg in range(ncg)]
                    for cg, (a, b) in enumerate(cgs):
                        for pr in range(KCH // 2):
                            nc.tensor.matmul(
                                s1[:, a:b],
                                w1sb[ci][:, 2 * pr : 2 * pr + 2, jl * P : (jl + 1) * P],
                                xt8[lc][:, 2 * pr : 2 * pr + 2, a:b],
                                start=(pr == 0),
                                stop=(pr == KCH // 2 - 1),
                                perf_mode=DR,
                            )
                    # s2 in single-bank tiles (pool depth 4): each half is
                    # released by its own product op, doubling the recycle
                    # slack that was stalling next-next-group matmuls
                    s2t = []
                    for cg, (a, b) in enumerate(cgs):
                        s2 = ppool2.tile([P, 512], F32, name=f"s2cg", tag="s2")
                        s2t.append(s2)
                        for pr in range(KCH // 2):
                            nc.tensor.matmul(
                                s2[:, 0 : b - a],
                                w2sb[ci][:, 2 * pr : 2 * pr + 2, jl * P : (jl + 1) * P],
                                xt8[lc][:, 2 * pr : 2 * pr + 2, a:b],
                                start=(pr == 0),
                                stop=(pr == KCH // 2 - 1),
                                perf_mode=DR,
                            )
                    e = epool.tile([P, 1024], F32)
                    nc.scalar.activation(
                        out=e[:, :lw], in_=s1[:, :lw], func=Exp,
                        scale=1.0 / W_SCALE,
                        accum_out=dall[:, 3 * j + lc : 3 * j + lc + 1],
                    )
                    for cg, (a, b) in enumerate(cgs):
                        idx = 5 * j + 2 * lc + cg
                        prod = spool.tile([P, 512], F32, name="prod", tag="prod")
                        nc.vector.scalar_tensor_tensor(
                            out=prod[:, 0 : b - a], in0=e[:, a:b],
                            scalar=1.0 / W_SCALE,
                            in1=s2t[cg][:, 0 : b - a], op0=mult, op1=mult,
                            accum_out=nall[:, idx : idx + 1],
                        )

            # Batched epilogue: segmented reduce over the 3 partials per j,
            # then out = numer/denom + b2 elementwise over [P, JCH].
            #
            # dall is written by ACT's accumulator-read micro-ops, and the
            # dependency tracker misses that cross-engine edge (observed: the
            # dred reduce issues with no ACT-side wait and intermittently
            # reads stale data). Force the ordering explicitly: a trailing
            # ACT copy is FIFO-ordered after every accumulator write, and a
            # DVE read of its (tracked) main output pins the DVE queue
            # behind it.
            probe = apool.tile([P, 1], F32, tag="probe")
            nc.scalar.activation(
                out=probe, in_=dall[:, 3 * JCH - 1 : 3 * JCH],
                func=mybir.ActivationFunctionType.Copy,
            )
            probe2 = apool.tile([P, 1], F32, tag="probe2")
            nc.vector.tensor_copy(probe2, probe)
            dred = apool.tile([P, JCH], F32, tag="dred")
            nred = apool.tile([P, JCH], F32, tag="nred")
            recip = apool.tile([P, JCH], F32, tag="recip")
            quot = apool.tile([P, JCH], F32, tag="quot")
            osb = apool.tile([P, JCH], F32, tag="osb")
            AX = mybir.AxisListType.X
            nc.vector.tensor_reduce(
                out=dred, in_=dall.rearrange("p (j l) -> p j l", l=3),
                axis=AX, op=add,
            )
            nc.vector.tensor_reduce(
                out=nred, in_=nall.rearrange("p (j l) -> p j l", l=5),
                axis=AX, op=add,
            )
            nc.vector.reciprocal(recip, dred)
            nc.vector.scalar_tensor_tensor(
                out=quot, in0=nred, scalar=1.0, in1=recip, op0=mult, op1=mult
            )
            nc.vector.scalar_tensor_tensor(
                out=osb, in0=quot, scalar=1.0, in1=b2sb, op0=mult, op1=add
            )
            nc.sync.dma_start(out=out[:], in_=osb)

    nc.compile()
    return nc


_NC_CACHE = {}


def _get_nc():
    if "nc" not in _NC_CACHE:
        _NC_CACHE["nc"] = build_nc()
    return _NC_CACHE["nc"]


def make_in_maps(x, W1, W2, b2):
    """Host-side prep: pad C, transpose + scale + fp8-cast weights, bf16 x."""
    x = np.asarray(x, dtype=np.float32)

    def prep_w(W):
        Wp = np.zeros((C_PAD, D), dtype=np.float32)
        Wp[:C] = np.asarray(W, dtype=np.float32)
        # [C_PAD, D] -> [P(d within chunk), KCH, C_PAD]
        return np.ascontiguousarray(
            Wp.T.reshape(KCH, P, C_PAD).transpose(1, 0, 2) * W_SCALE
        ).astype(FP8_NP)

    w1c, w2c = prep_w(W1), prep_w(W2)
    b2p = np.zeros((C_PAD,), dtype=np.float32)
    b2p[:C] = np.asarray(b2, dtype=np.float32)
    b2c = np.ascontiguousarray(b2p.reshape(JCH, P).T)

    in_maps = []
    for i in range(N_CORES):
        xc = np.ascontiguousarray(
            x[i].reshape(KCH, P, L).transpose(1, 0, 2)
        ).astype(BF16_NP)
        in_maps.append({"x": xc, "w1t": w1c, "w2t": w2c, "b2s": b2c})
    return in_maps


def gather_out(results):
    """results: list (per core) of {'out': [P, JCH]} -> full [B, C]."""
    parts = [
        np.asarray(r["out"], dtype=np.float32).T.reshape(C_PAD)[:C]
        for r in results
    ]
    return np.stack(parts, axis=0)


def kernel(x, W1, W2, b2):
    nc = _get_nc()
    in_maps = make_in_maps(x, W1, W2, b2)
    res = run_bass_kernel_spmd(nc, in_maps, list(range(N_CORES)))
    return gather_out(res.results)


# revision 7
# speedup vs baseline: 1.0374x; 1.0032x over previous
"""CAML attention kernel for Trainium2 (8 NeuronCores, SPMD over batch).

Reference computation:
    xt      = tanh(x)                      # [B, D, L]
    scores  = einsum('cd,bdl->bcl', W1, xt)
    weights = softmax(scores, axis=l)
    weighted= einsum('bcl,bdl->bcd', weights, xt)
    out     = einsum('cd,bcd->bc', W2, weighted) + b2

Key identity: the final contraction commutes with the softmax weighted sum,
so with s2 = einsum('cd,bdl->bcl', W2, xt):
    out[b,c] = (sum_l exp(s1)*s2) / (sum_l exp(s1)) + b2
(|s1| <= 512*max|W1| ~ 13, so exp without max-subtraction is safe in fp32.)

Design (vs the 637us fp16 C-sharded baseline; measured ~327us):
  * Batch-sharded: core i computes batch i with the full class range
    (C padded 8930 -> 8960 = 70*128). 8x less x DMA + tanh per core, and
    jch drops 72 -> 70 vs the C_PAD=9216 C-sharding.
  * Both matmuls in fp8-e4m3 DoubleRow (contraction 256/instr, issues at
    the plain N/2.4GHz streaming rate): weights are scaled by 16 into
    e4m3's normal range; exp() compensates with scale=1/16 and the
    numerator product with scalar=1/16. tanh() writes fp8 directly from
    ACT (no DVE cast pass). Measured HW rel err 1.3e-2 (< 2e-2 gate).
  * s1 PSUM groups of 1024 cols (2 banks, double-buffered; L =
    1024+1024+452): 3 exp ops per j with ACT-side denominator accum —
    ACT was the 100%-busy bottleneck in a 500-col-group fp8 experiment.
    s2 lands in single-bank [P,512] tiles (pool depth 4), each released
    by its own product op: the deeper recycle removes the once-per-j
    matmul stalls that depth-2 1024-col s2 buffering caused.
  * Batched epilogue: per-group numer/denom partials land in persistent
    [P,350]/[P,210] accumulators via DVE/ACT accum_out; one segmented
    reduce + reciprocal + 2 elementwise ops at the end replace 4-6 small
    ops per j (which previously kept ACT saturated).
"""

import numpy as np
import ml_dtypes

import concourse.bacc as bacc
import concourse.tile as tile
from concourse import mybir
from concourse.bass_utils import run_bass_kernel_spmd

B, D, L, C = 8, 512, 2500, 8930
N_CORES = 8
P = 128

C_PAD = 8960                 # next multiple of 128 above C
JCH = C_PAD // P             # 70 class chunks per core
KCH = D // P                 # 4 contraction chunks (2 DoubleRow pairs)
JCW = [4, 11, 11, 11, 11, 11, 11]   # j's per weight-DMA chunk (small first)
JCO = [0, 4, 15, 26, 37, 48, 59]     # chunk offsets
NJC = len(JCW)
LCS = [(0, 1024), (1024, 1024), (2048, 452)]   # (start, len) PSUM groups
LCW = [1024, 1024, 512]      # xt8 tile row strides (16B-aligned for fp8 rhs)

F32 = mybir.dt.float32
BF16 = mybir.dt.bfloat16
FP8 = mybir.dt.float8e4
FP8_NP = mybir.dt.np(mybir.dt.float8e4)   # ml_dtypes.float8_e4m3
BF16_NP = ml_dtypes.bfloat16

W_SCALE = 16.0               # lift ~U(-0.025, 0.025) weights into e4m3 normals
DR = mybir.MatmulPerfMode.DoubleRow


def build_nc():
    """Emit the per-core program. All cores run the same NEFF (SPMD)."""
    nc = bacc.Bacc("TRN2", target_bir_lowering=False, debug=False)

    x = nc.dram_tensor("x", [P, KCH, L], BF16, kind="ExternalInput")
    w1t = nc.dram_tensor("w1t", [P, KCH, C_PAD], FP8, kind="ExternalInput")
    w2t = nc.dram_tensor("w2t", [P, KCH, C_PAD], FP8, kind="ExternalInput")
    b2s = nc.dram_tensor("b2s", [P, JCH], F32, kind="ExternalInput")
    out = nc.dram_tensor("out", [P, JCH], F32, kind="ExternalOutput")

    Exp = mybir.ActivationFunctionType.Exp
    Tanh = mybir.ActivationFunctionType.Tanh
    mult = mybir.AluOpType.mult
    add = mybir.AluOpType.add

    with tile.TileContext(nc) as tc:
        with (
            tc.tile_pool(name="wts", bufs=1) as wpool,
            tc.tile_pool(name="xraw", bufs=1) as xpool,
            tc.tile_pool(name="xt8", bufs=1) as xtpool,
            tc.tile_pool(name="ps1", bufs=2, space="PSUM") as ppool1,
            tc.tile_pool(name="ps2", bufs=4, space="PSUM") as ppool2,
            tc.tile_pool(name="etile", bufs=4) as epool,
            tc.tile_pool(name="prod", bufs=3) as spool,
            tc.tile_pool(name="acc", bufs=1) as apool,
        ):
            # Weight SBUF tiles, one per DMA chunk so early matmuls only
            # depend on the first chunk's arrival.
            w1sb = [wpool.tile([P, KCH, JCW[ci] * P], FP8, tag=f"w1_{ci}", name=f"w1sb{ci}") for ci in range(NJC)]
            w2sb = [wpool.tile([P, KCH, JCW[ci] * P], FP8, tag=f"w2_{ci}", name=f"w2sb{ci}") for ci in range(NJC)]
            b2sb = wpool.tile([P, JCH], F32, tag="b2")

            # x chunks (bf16) and tanh(x) in fp8; chunk 0 split in halves
            # so the very first matmuls wait on only 256 KB of x
            XCH = [(0, 512), (512, 512), (1024, 1024), (2048, 452)]
            xraw = [xpool.tile([P, KCH, lw], BF16, tag=f"xr_{i}", name=f"xraw{i}") for i, (_, lw) in enumerate(XCH)]
            xt8 = [xtpool.tile([P, KCH, LCW[i]], FP8, tag=f"xt_{i}", name=f"xt8_{i}") for i in range(3)]

            # partial accumulators: denom col j*3 + lc, numer col j*5 + cg
            dall = apool.tile([P, 3 * JCH], F32, tag="dall")
            nall = apool.tile([P, 5 * JCH], F32, tag="nall")

            # DMA order = first-consumption order on the single sync queue
            def wslice(ci):
                return slice(JCO[ci] * P, (JCO[ci] + JCW[ci]) * P)

            nc.sync.dma_start(out=xraw[0], in_=x[:, :, 0:512])
            nc.sync.dma_start(out=w1sb[0], in_=w1t[:, :, wslice(0)])
            nc.sync.dma_start(out=w2sb[0], in_=w2t[:, :, wslice(0)])
            nc.sync.dma_start(out=xraw[1], in_=x[:, :, 512:1024])
            nc.sync.dma_start(out=xraw[2], in_=x[:, :, 1024:2048])
            nc.sync.dma_start(out=xraw[3], in_=x[:, :, 2048:2500])
            for ci in range(1, NJC):
                nc.sync.dma_start(out=w1sb[ci], in_=w1t[:, :, wslice(ci)])
                nc.sync.dma_start(out=w2sb[ci], in_=w2t[:, :, wslice(ci)])
            nc.sync.dma_start(out=b2sb, in_=b2s[:])

            for j in range(JCH):
                ci = next(i for i in range(NJC - 1, -1, -1) if JCO[i] <= j)
                jl = j - JCO[ci]
                for lc, (ls, lw) in enumerate(LCS):
                    if j == 0:
                        # tanh for this l-chunk, emitted right before its
                        # first consumer so ACT doesn't head-block on the
                        # later x DMAs.
                        for xi, (xs, xw) in enumerate(XCH):
                            if ls <= xs < ls + lw:
                                nc.scalar.activation(
                                    out=xt8[lc][:, :, xs - ls : xs - ls + xw],
                                    in_=xraw[xi], func=Tanh,
                                )
                    s1 = ppool1.tile([P, 1024], F32)
                    ncg = (lw + 511) // 512
                    cgs = [(512 * cg, min(512 * (cg + 1), lw)) for cg in range(ncg)]
                    for cg, (a, b) in enumerate(cgs):
                        for pr in range(KCH // 2):
                            nc.tensor.matmul(
                                s1[:, a:b],
                                w1sb[ci][:, 2 * pr : 2 * pr + 2, jl * P : (jl + 1) * P],
                                xt8[lc][:, 2 * pr : 2 * pr + 2, a:b],
                                start=(pr == 0),
                                stop=(pr == KCH // 2 - 1),
                                perf_mode=DR,
                            )
                    # s2 in single-bank tiles (pool depth 4): each half is
                    # released by its own product op, doubling the recycle
                    # slack that was stalling next-next-group matmuls
                    s2t = []
                    for cg, (a, b) in enumerate(cgs):
                        s2 = ppool2.tile([P, 512], F32, name=f"s2cg", tag="s2")
                        s2t.append(s2)
                        for pr in range(KCH // 2):
                            nc.tensor.matmul(
                                s2[:, 0 : b - a],
                                w2sb[ci][:, 2 * pr : 2 * pr + 2, jl * P : (jl + 1) * P],
                                xt8[lc][:, 2 * pr : 2 * pr + 2, a:b],
                                start=(pr == 0),
                                stop=(pr == KCH // 2 - 1),
                                perf_mode=DR,
                            )
                    e = epool.tile([P, 1024], F32)
                    nc.scalar.activation(
                        out=e[:, :lw], in_=s1[:, :lw], func=Exp,
                        scale=1.0 / W_SCALE,
                        accum_out=dall[:, 3 * j + lc : 3 * j + lc + 1],
                    )
                    for cg, (a, b) in enumerate(cgs):
                        idx = 5 * j + 2 * lc + cg
                        prod = spool.tile([P, 512], F32, name="prod", tag="prod")
                        nc.vector.scalar_tensor_tensor(
                            out=prod[:, 0 : b - a], in0=e[:, a:b],
                            scalar=1.0 / W_SCALE,
                            in1=s2t[cg][:, 0 : b - a], op0=mult, op1=mult,
                            accum_out=nall[:, idx : idx + 1],
                        )

            # Batched epilogue: segmented reduce over the 3 partials per j,
            # then out = numer/denom + b2 elementwise over [P, JCH].
            #
            # dall is written by ACT's accumulator-read micro-ops, and the
            # dependency tracker misses that cross-engine edge (observed: the
            # dred reduce issues with no ACT-side wait and intermittently
            # reads stale data). Force the ordering explicitly: a trailing
            # ACT copy is FIFO-ordered after every accumulator write, and a
            # DVE read of its (tracked) main output pins the DVE queue
            # behind it.
            probe = apool.tile([P, 1], F32, tag="probe")
            nc.scalar.activation(
                out=probe, in_=dall[:, 3 * JCH - 1 : 3 * JCH],
                func=mybir.ActivationFunctionType.Copy,
            )
            probe2 = apool.tile([P, 1], F32, tag="probe2")
            nc.vector.tensor_copy(probe2, probe)
            dred = apool.tile([P, JCH], F32, tag="dred")
            nred = apool.tile([P, JCH], F32, tag="nred")
            recip = apool.tile([P, JCH], F32, tag="recip")
            quot = apool.tile([P, JCH], F32, tag="quot")
            osb = apool.tile([P, JCH], F32, tag="osb")
            AX = mybir.AxisListType.X
            nc.vector.tensor_reduce(
                out=dred, in_=dall.rearrange("p (j l) -> p j l", l=3),
                axis=AX, op=add,
            )
            nc.vector.tensor_reduce(
                out=nred, in_=nall.rearrange("p (j l) -> p j l", l=5),
                axis=AX, op=add,
            )
            nc.vector.reciprocal(recip, dred)
            nc.vector.scalar_tensor_tensor(
                out=quot, in0=nred, scalar=1.0, in1=recip, op0=mult, op1=mult
            )
            nc.vector.scalar_tensor_tensor(
                out=osb, in0=quot, scalar=1.0, in1=b2sb, op0=mult, op1=add
            )
            nc.sync.dma_start(out=out[:], in_=osb)

    nc.compile()
    return nc


_NC_CACHE = {}


def _get_nc():
    if "nc" not in _NC_CACHE:
        _NC_CACHE["nc"] = build_nc()
    return _NC_CACHE["nc"]


def make_in_maps(x, W1, W2, b2):
    """Host-side prep: pad C, transpose + scale + fp8-cast weights, bf16 x."""
    x = np.asarray(x, dtype=np.float32)

    def prep_w(W):
        Wp = np.zeros((C_PAD, D), dtype=np.float32)
        Wp[:C] = np.asarray(W, dtype=np.float32)
        # [C_PAD, D] -> [P(d within chunk), KCH, C_PAD]
        return np.ascontiguousarray(
            Wp.T.reshape(KCH, P, C_PAD).transpose(1, 0, 2) * W_SCALE
        ).astype(FP8_NP)

    w1c, w2c = prep_w(W1), prep_w(W2)
    b2p = np.zeros((C_PAD,), dtype=np.float32)
    b2p[:C] = np.asarray(b2, dtype=np.float32)
    b2c = np.ascontiguousarray(b2p.reshape(JCH, P).T)

    in_maps = []
    for i in range(N_CORES):
        xc = np.ascontiguousarray(
            x[i].reshape(KCH, P, L).transpose(1, 0, 2)
        ).astype(BF16_NP)
        in_maps.append({"x": xc, "w1t": w1c, "w2t": w2c, "b2s": b2c})
    return in_maps


def gather_out(results):
    """results: list (per core) of {'out': [P, JCH]} -> full [B, C]."""
    parts = [
        np.asarray(r["out"], dtype=np.float32).T.reshape(C_PAD)[:C]
        for r in results
    ]
    return np.stack(parts, axis=0)


def kernel(x, W1, W2, b2):
    nc = _get_nc()
    in_maps = make_in_maps(x, W1, W2, b2)
    res = run_bass_kernel_spmd(nc, in_maps, list(range(N_CORES)))
    return gather_out(res.results)
